# revision 1
# baseline (speedup 1.0000x reference)
"""Trainium2 Bass kernel v3 for nn_Decoder_48052094107929 (moe_routing).

Data-parallel over 8 NeuronCores: batch B=8192 split into 8 shards of 1024
tokens; weights replicated. Differences vs v2:

  - no dma_start_transpose (1.2us/call engine occupancy serialized phase 0);
    all transposes on PE in bf16 (62ns each)
  - phase 0 split into grouped loops: gating (all 8 Exp together) then
    layernorm (all 8 Sqrt together) -> 2 ACT table loads instead of 19
  - qkv accumulation runs all x-side (kv) steps before ny-side (q) steps so
    qkv starts as soon as gating chunk 0 lands, ~40us before layernorm done
  - both sc streams (q and kv side) produced on the vector engine (gpsimd
    measured 56 G elem/s, was pacing the PE)
  - fc1 also fp8 DoubleRow (x64 weights; gelu eviction applies 1/64 descale)
  - attention/fc1 interleaved in 4-tile halves (2 Gelu<->Exp table switches)
"""

import numpy as np
import ml_dtypes

import concourse.bass as bass
import concourse.mybir as mybir
import concourse.tile as tile
from concourse.bass_utils import run_bass_kernel_spmd
from concourse.masks import make_identity

# ---- problem constants (hardcoded per harness contract) ----
B = 8192
DIM = 1024
E = 4
H = 4
HD = DIM // H          # 256
SCALE = HD ** -0.5
HID = 4 * DIM          # 4096
EPS = 1e-5
NCORES = 8
B_C = B // NCORES      # 1024 tokens per core

F32 = mybir.dt.float32
BF16 = mybir.dt.bfloat16
FP8 = mybir.dt.float8e4
AX = mybir.AxisListType
OP = mybir.AluOpType
AF = mybir.ActivationFunctionType
DR = mybir.MatmulPerfMode.DoubleRow

KD = DIM // 128        # 8 d-tiles
PAIRS = KD // 2        # 4 DoubleRow k-pairs
MH = HID // 128        # 32 hidden tiles
T = B_C // 128         # 8 token tiles
CHUNK = 512
NCH = B_C // CHUNK     # 2
NEG_BIG = -1.0e30
WSC = 64.0             # fp8 weight prescale (power of 2, exact to invert)

QKV_FP8 = True
FC1_FP8 = True
FC2_FP8 = True


def bf(a):
    return np.ascontiguousarray(a.astype(ml_dtypes.bfloat16))


def f32(a):
    return np.ascontiguousarray(a.astype(np.float32))


def fp8(a):
    return np.ascontiguousarray(a.astype(ml_dtypes.float8_e4m3fn))


def prep_weights(Wg, bg, Wqkv, Wp, bp, g1, bn1, g2, bn2, W1, bm1, W2, bm2):
    """Host-side, input-independent weight layout transforms."""
    Wq = Wqkv[:, :DIM, :]                        # [E, DIM, DIM] (f, d)
    Wk = Wqkv[:, DIM:2 * DIM, :]
    Wv = Wqkv[:, 2 * DIM:, :]
    Wqp = Wq * g1[None, None, :]                 # fold norm1 gamma into cols
    bq = np.einsum("efd,d->ef", Wq, bn1)         # [E, DIM] bias from norm1 beta
    Wkvs = Wk + Wv                               # aliasing bug: k+v share weights

    # [E, PAIRS, 128p, M, 2, 128f]: p-major DMA runs, contiguous lhsT slices
    WqT = (Wqp.transpose(0, 2, 1).reshape(E, PAIRS, 2, 128, KD, 128)
           .transpose(0, 1, 3, 4, 2, 5))
    WkvT = (Wkvs.transpose(0, 2, 1).reshape(E, PAIRS, 2, 128, KD, 128)
            .transpose(0, 1, 3, 4, 2, 5))
    WpT = Wp.T.reshape(KD, 128, DIM)
    if FC2_FP8:
        WpT = WpT * WSC          # shares the fc2 PSUM descale (bf16-exact)
    W1p = W1 * g2[None, :]
    bm1p = bm1 + W1 @ bn2
    W1T = W1p.T.reshape(PAIRS, 2, 128, HID)
    if FC2_FP8:
        # [16 kh-pairs, 128p, 8m, 2, 128f]: p-major, contiguous lhsT slices
        W2T = (W2.T.reshape(MH // 2, 2, 128, KD, 128)
               .transpose(0, 2, 3, 1, 4))
    else:
        W2T = W2.T.reshape(MH, 128, DIM)
    WgT = Wg.T.reshape(KD, 128, E)
    bqT = bq.reshape(E, KD, 128)

    out = {
        "wp": bf(WpT), "wg": f32(WgT),
        "bgv": f32(bg.reshape(1, E)),
        "bgp": f32(bg.reshape(E, 1)),
        "bm1v": f32(bm1p.reshape(MH, 128).T),                  # [128,32]
        "bpb2": f32((bp + bm2).reshape(KD, 128).T),            # [128,8]
    }
    if QKV_FP8:
        out["wq"] = fp8(WqT * WSC)
        out["wkv"] = fp8(WkvT * WSC)
        out["bq"] = bf(bqT * WSC)
    else:
        out["wq"] = bf(WqT)
        out["wkv"] = bf(WkvT)
        out["bq"] = bf(bqT)
    if FC1_FP8:
        out["w1"] = fp8(W1T * WSC)
    else:
        out["w1"] = bf(W1T)
    if FC2_FP8:
        out["w2"] = fp8(W2T * WSC)
    else:
        out["w2"] = bf(W2T)
    return out


def build_kernel(b_c=B_C):
    """Build the Bass module for one core processing b_c tokens."""
    nc = bass.Bass("TRN2", target_bir_lowering=False, debug=False)
    qdt = FP8 if QKV_FP8 else BF16
    fdt = FP8 if FC1_FP8 else BF16
    qdesc = (1.0 / WSC) if QKV_FP8 else 1.0
    fdesc = (1.0 / WSC) if FC1_FP8 else 1.0

    # ---- DRAM tensors ----
    xT_d = nc.dram_tensor("xT", [DIM, b_c], F32, kind="ExternalInput")
    y_d = nc.dram_tensor("y", [b_c, DIM], F32, kind="ExternalInput")
    yT_d = nc.dram_tensor("yT", [DIM, b_c], F32, kind="ExternalInput")
    wq_d = nc.dram_tensor("wq", [E, PAIRS, 128, KD, 2, 128], qdt,
                          kind="ExternalInput")
    wkv_d = nc.dram_tensor("wkv", [E, PAIRS, 128, KD, 2, 128], qdt,
                          kind="ExternalInput")
    wp_d = nc.dram_tensor("wp", [KD, 128, DIM], BF16, kind="ExternalInput")
    w1_d = nc.dram_tensor("w1", [PAIRS, 2, 128, HID], fdt, kind="ExternalInput")
    if FC2_FP8:
        w2_d = nc.dram_tensor("w2", [MH // 2, 128, KD, 2, 128], FP8,
                              kind="ExternalInput")
    else:
        w2_d = nc.dram_tensor("w2", [MH, 128, DIM], BF16,
                              kind="ExternalInput")
    wg_d = nc.dram_tensor("wg", [KD, 128, E], F32, kind="ExternalInput")
    bq_d = nc.dram_tensor("bq", [E, KD, 128], BF16, kind="ExternalInput")
    bg_d = nc.dram_tensor("bgv", [1, E], F32, kind="ExternalInput")
    bgp_d = nc.dram_tensor("bgp", [E, 1], F32, kind="ExternalInput")
    bm1_d = nc.dram_tensor("bm1v", [128, MH], F32, kind="ExternalInput")
    bpb2_d = nc.dram_tensor("bpb2", [128, KD], F32, kind="ExternalInput")
    outT_d = nc.dram_tensor("outT", [DIM, b_c], F32, kind="ExternalOutput")
    csc_d = nc.dram_tensor("cscratch", [E, b_c], BF16, kind="Internal")

    xT_r = xT_d.ap().rearrange("(k p) b -> p k b", p=128)
    y_r = y_d.ap().rearrange("(t p) d -> t p d", p=128)
    yT_r = yT_d.ap().rearrange("(k p) b -> p k b", p=128)
    outT_r = outT_d.ap().rearrange("(k p) b -> k p b", p=128)

    from contextlib import ExitStack

    with tile.TileContext(nc) as tc, ExitStack() as ctx0:
        consts = ctx0.enter_context(tc.tile_pool(name="consts", bufs=1))
        ident_bf = consts.tile([128, 128], BF16)
        make_identity(nc, ident_bf)
        ident_f = consts.tile([128, 128], F32)
        make_identity(nc, ident_f)
        eps_t = consts.tile([128, 1], F32)
        nc.vector.memset(eps_t, EPS)
        bgp_sb = consts.tile([4, 1], F32)
        nc.sync.dma_start(out=bgp_sb, in_=bgp_d.ap())
        wg_sb = consts.tile([128, KD, E], F32)
        nc.sync.dma_start(out=wg_sb, in_=wg_d.ap().rearrange("k p e -> p k e"))
        bq_sb = consts.tile([4, KD, 128], BF16)
        nc.sync.dma_start(out=bq_sb, in_=bq_d.ap())
        bm1_sb = consts.tile([128, MH], F32)
        nc.sync.dma_start(out=bm1_sb, in_=bm1_d.ap())
        bpb2_sb = consts.tile([128, KD], F32)
        nc.sync.dma_start(out=bpb2_sb, in_=bpb2_d.ap())

        nyT_p = ctx0.enter_context(tc.tile_pool(name="nyT", bufs=1))
        nyT = nyT_p.tile([128, KD, b_c], BF16)
        if FC1_FP8:
            nyT8_p = ctx0.enter_context(tc.tile_pool(name="nyT8", bufs=1))
            nyT8 = nyT8_p.tile([128, PAIRS, NCH, 2, CHUNK], FP8)
        xTb_p = ctx0.enter_context(tc.tile_pool(name="xTb", bufs=1))
        xTb = xTb_p.tile([128, KD, b_c], BF16)
        sT_p = ctx0.enter_context(tc.tile_pool(name="sT", bufs=1))
        sT = sT_p.tile([128, KD, b_c], BF16)
        oT_p = ctx0.enter_context(tc.tile_pool(name="oT", bufs=1))
        oT = oT_p.tile([128, KD, b_c], BF16)
        cb_p = ctx0.enter_context(tc.tile_pool(name="cb", bufs=1))
        cb = cb_p.tile([128, E, b_c], BF16)
        crows_p = ctx0.enter_context(tc.tile_pool(name="crows", bufs=1))
        crows = crows_p.tile([4, b_c], BF16)

        # ---------- phase 0: loads, gating (loop A), layernorm (loop B) ----
        with ExitStack() as p0:
            xTf_p = p0.enter_context(tc.tile_pool(name="xTf", bufs=1))
            xTf = xTf_p.tile([128, KD, b_c], F32)
            gsm = p0.enter_context(tc.tile_pool(name="gsm", bufs=8))
            g_ps = p0.enter_context(
                tc.tile_pool(name="g_ps", bufs=2, space="PSUM"))
            cr_ps = p0.enter_context(
                tc.tile_pool(name="cr_ps", bufs=1, space="PSUM"))
            crows_ps = cr_ps.tile([4, b_c], F32)

            # x^T loads: 16 calls (kd x column-half) spread over DMA queues;
            # chunk-0 halves first so gating tiles 0-3 start earliest
            for half in range(NCH):
                csl = slice(half * CHUNK, (half + 1) * CHUNK)
                for kd in range(KD):
                    nc.sync.dma_start(out=xTf[:, kd, csl],
                                      in_=xT_r[:, kd, csl])
            # x^T bf16 tiles 0-3 early on vector (feeds gpsimd x-side sc)
            for t4 in range(4):
                t4sl = slice(t4 * 128, (t4 + 1) * 128)
                nc.vector.tensor_copy(out=xTb[:, :, t4sl],
                                      in_=xTf[:, :, t4sl])

            # flipped gating: glog^T[e, tok] = wg^T x^T + bg (bg is a
            # per-partition scalar in this layout)
            glT_sb = gsm.tile([4, b_c], F32, tag="glT", bufs=1)

            # ---- loop A: gating per tile (all Exp together: 1 ACT table) --
            for t in range(T):
                tsl = slice(t * 128, (t + 1) * 128)
                if t % (CHUNK // 128) == 0:
                    gch = t // (CHUNK // 128)
                    gsl = slice(gch * CHUNK, (gch + 1) * CHUNK)
                    glT = g_ps.tile([4, CHUNK], F32, tag="glT_ps")
                    for kd in range(KD):
                        nc.tensor.matmul(glT, wg_sb[:, kd, :],
                                         xTf[:, kd, gsl],
                                         start=(kd == 0), stop=(kd == KD - 1))
                    nc.vector.tensor_scalar(out=glT_sb[:, gsl], in0=glT,
                                            scalar1=bgp_sb, scalar2=None,
                                            op0=OP.add)
                glps = g_ps.tile([128, E], F32, tag="gps")
                nc.tensor.transpose(glps, glT_sb[:, tsl], ident_f[:4, :4])
                glog = glps
                gm = gsm.tile([128, 1], F32, tag="gm")
                nc.vector.tensor_reduce(out=gm, in_=glog, axis=AX.X, op=OP.max)
                ngm = gsm.tile([128, 1], F32, tag="ngm")
                nc.vector.tensor_scalar_mul(ngm, gm, -1.0)
                gexp = gsm.tile([128, E], F32, tag="gexp")
                gden = gsm.tile([128, 1], F32, tag="gden")
                nc.scalar.activation(out=gexp, in_=glog, func=AF.Exp,
                                     bias=ngm, scale=1.0, accum_out=gden)
                grec = gsm.tile([128, 1], F32, tag="grec")
                nc.vector.reciprocal(out=grec, in_=gden)
                gate = gsm.tile([128, E], F32, tag="gate")
                nc.vector.tensor_scalar_mul(gate, gexp, grec)
                # top-2 mask: keep entries >= second max
                m1 = gsm.tile([128, 1], F32, tag="m1")
                nc.vector.tensor_reduce(out=m1, in_=gate, axis=AX.X, op=OP.max)
                eq1 = gsm.tile([128, E], F32, tag="eq1")
                nc.vector.tensor_scalar(out=eq1, in0=gate, scalar1=m1,
                                        scalar2=None, op0=OP.is_equal)
                msk = gsm.tile([128, E], F32, tag="msk")
                nc.vector.scalar_tensor_tensor(out=msk, in0=eq1,
                                               scalar=NEG_BIG, in1=gate,
                                               op0=OP.mult, op1=OP.add)
                m2 = gsm.tile([128, 1], F32, tag="m2")
                nc.vector.tensor_reduce(out=m2, in_=msk, axis=AX.X, op=OP.max)
                keep = gsm.tile([128, E], F32, tag="keep")
                nc.vector.tensor_scalar(out=keep, in0=gate, scalar1=m2,
                                        scalar2=None, op0=OP.is_ge)
                c_tok = gsm.tile([128, E], F32, tag="c_tok")
                nc.vector.tensor_mul(c_tok, gate, keep)
                # c -> [4, tokens] row layout; stage to DRAM for broadcast
                nc.tensor.transpose(crows_ps[:, tsl], c_tok, ident_f)
                nc.vector.tensor_copy(out=crows[:, tsl], in_=crows_ps[:, tsl])
                nc.gpsimd.dma_start(out=csc_d.ap()[:, tsl],
                                    in_=crows[:, tsl])
                if ((t + 1) * 128) % CHUNK == 0:
                    ch = ((t + 1) * 128) // CHUNK - 1
                    csl = slice(ch * CHUNK, (ch + 1) * CHUNK)
                    for e in range(E):
                        nc.gpsimd.dma_start(
                            out=cb[:, e, csl],
                            in_=csc_d.ap()[e:e + 1, csl]
                            .to_broadcast([128, CHUNK]))

            # x^T bf16 tiles 4-7 on vector (gpsimd is pacing ch0 x-side sc)
            for t4 in range(4, T):
                t4sl = slice(t4 * 128, (t4 + 1) * 128)
                nc.vector.tensor_copy(out=xTb[:, :, t4sl],
                                      in_=xTf[:, :, t4sl])

        # ---------- phase 1: qkv expert matmuls (fp8 DoubleRow) ----------
        # x-side (kv) accumulation steps first: they only need gating + x^T,
        # so the PE starts ~40us before layernorm finishes producing nyT.
        with ExitStack() as p1:
            wstr = p1.enter_context(tc.tile_pool(name="wstr", bufs=8))
            scl = p1.enter_context(tc.tile_pool(name="scl", bufs=8))
            ypool = p1.enter_context(tc.tile_pool(name="yin", bufs=1))
            nrm = p1.enter_context(tc.tile_pool(name="nrm", bufs=3))
            stat = p1.enter_context(tc.tile_pool(name="stat", bufs=6))
            qk_ps = p1.enter_context(
                tc.tile_pool(name="qk_ps", bufs=1, space="PSUM"))
            yts = []
            for t in range(T):
                yt = ypool.tile([128, DIM], F32, tag=f"yt{t}")
                nc.sync.dma_start(out=yt, in_=y_r[t])
                yts.append(yt)

            def loop_b_tile(t):
                # layernorm(y) tile: stats + normalize (vector), sqrt
                # (scalar), feature-major via DMA-xbar (scalar queue)
                if True:
                    tsl = slice(t * 128, (t + 1) * 128)
                    yt = yts[t]
                    st6 = stat.tile([128, 2, 6], F32, tag="st6")
                    yv = yt.rearrange("p (s d) -> p s d", s=2)
                    for s in range(2):
                        nc.vector.bn_stats(out=st6[:, s, :], in_=yv[:, s, :])
                    mv = stat.tile([128, 2], F32, tag="mv")
                    nc.vector.bn_aggr(out=mv, in_=st6)
                    sd = stat.tile([128, 1], F32, tag="sd")
                    nc.scalar.activation(out=sd, in_=mv[:, 1:2], func=AF.Sqrt,
                                         bias=eps_t, scale=1.0)
                    rstd = stat.tile([128, 1], F32, tag="rstd")
                    nc.vector.reciprocal(out=rstd, in_=sd)
                    ny = nrm.tile([128, DIM], BF16, tag="ny")
                    nc.vector.tensor_scalar(out=ny, in0=yt,
                                            scalar1=mv[:, 0:1],
                                            scalar2=rstd, op0=OP.subtract,
                                            op1=OP.mult)
                    nc.scalar.dma_start_transpose(
                        out=nyT[:, :, tsl], in_=ny)
                    if FC1_FP8:
                        TPC = CHUNK // 128
                        csl8 = slice((t % TPC) * 128, (t % TPC + 1) * 128)
                        nc.gpsimd.tensor_copy(
                            out=nyT8[:, :, t // TPC, :, csl8],
                            in_=nyT[:, :, tsl].rearrange(
                                "p (pr two) n -> p pr two n", two=2))

            for ch in range(NCH):
                csl = slice(ch * CHUNK, (ch + 1) * CHUNK)
                ps = [qk_ps.tile([128, CHUNK], F32, tag=f"qk{m}",
                                 name=f"qk{m}_{ch}") for m in range(KD)]
                step = 0
                if QKV_FP8:
                    for which, (w_d2, act) in (
                            (1, (wkv_d, xTb)), (0, (wq_d, nyT))):
                        # ch1 x-side sc on gpsimd; everything else vector
                        seng = nc.gpsimd if (which == 1 and ch == 1) \
                            else nc.vector
                        for e in range(E):
                            for pair in range(PAIRS):
                                wt = wstr.tile([128, KD, 2, 128], FP8,
                                               tag="wt")
                                wsrc = w_d2.ap()[e, pair]
                                for q in range(4):
                                    qs = slice(q * 2, (q + 1) * 2)
                                    nc.sync.dma_start(out=wt[:, qs],
                                                      in_=wsrc[:, qs])
                                sc = scl.tile([128, 2, CHUNK], FP8,
                                              tag=f"sc{which}{ch}", bufs=8)
                                seng.tensor_mul(sc[:, 0, :],
                                                act[:, 2 * pair, csl],
                                                cb[:, e, csl])
                                seng.tensor_mul(sc[:, 1, :],
                                                act[:, 2 * pair + 1, csl],
                                                cb[:, e, csl])
                                for m in range(KD):
                                    nc.tensor.matmul(
                                        ps[m], wt[:, m],
                                        sc, start=(step == 0), stop=False,
                                        perf_mode=DR)
                                step += 1
                                if which == 1 and ch == 0 and step % 2 == 0 \
                                        and step // 2 <= T:
                                    loop_b_tile(step // 2 - 1)
                else:
                    for which, (w_d2, act) in (
                            (1, (wkv_d, xTb)), (0, (wq_d, nyT))):
                        for e in range(E):
                            for pair in range(PAIRS):
                                for two in range(2):
                                    kd = 2 * pair + two
                                    wt = wstr.tile([128, DIM], BF16, tag="wt")
                                    wsrc = w_d2.ap()[e, pair].rearrange(
                                        "p m two f -> p (m f) two")[:, :, two]
                                    for q in range(2):
                                        qs = slice(q * 512, (q + 1) * 512)
                                        nc.sync.dma_start(out=wt[:, qs],
                                                          in_=wsrc[:, qs])
                                    sc = scl.tile([128, CHUNK], BF16,
                                                  tag=f"sc{which}")
                                    nc.vector.tensor_mul(sc, act[:, kd, csl],
                                                         cb[:, e, csl])
                                    for m in range(KD):
                                        nc.tensor.matmul(
                                            ps[m],
                                            wt[:, m * 128:(m + 1) * 128],
                                            sc, start=(step == 0), stop=False)
                                    step += 1
                # bias step: sum_e c[e,t] * bq[e,f] (bf16, plain mode)
                for m in range(KD):
                    nc.tensor.matmul(ps[m], bq_sb[:, m, :], crows[:, csl],
                                     start=False, stop=True)
                for m in range(KD):
                    nc.scalar.activation(out=sT[:, m, csl], in_=ps[m],
                                         func=AF.Copy, scale=qdesc)

        # ---------- phases 2+3: attention (DVE) + fc1 (PE), interleaved ----
        with ExitStack() as ctxb:
            yT_p = ctxb.enter_context(tc.tile_pool(name="yTsb", bufs=1))
            yT_sb = yT_p.tile([128, KD, b_c], F32)
            hT_p = ctxb.enter_context(tc.tile_pool(name="hT", bufs=1))
            if FC2_FP8:
                hT = hT_p.tile([128, MH // 2, NCH, 2, CHUNK], FP8)
            else:
                hT = hT_p.tile([128, MH, b_c], BF16)
            with ExitStack() as p2:
                stok_p = p2.enter_context(tc.tile_pool(name="stok", bufs=4))
                otok_p = p2.enter_context(tc.tile_pool(name="otok", bufs=4))
                asm = p2.enter_context(tc.tile_pool(name="asm", bufs=8))
                scr = p2.enter_context(tc.tile_pool(name="scr", bufs=4))
                w1str = p2.enter_context(tc.tile_pool(name="w1str", bufs=6))
                at_ps = p2.enter_context(
                    tc.tile_pool(name="at_ps", bufs=4, space="PSUM"))
                f1_ps = p2.enter_context(
                    tc.tile_pool(name="f1_ps", bufs=4, space="PSUM"))

                for kd in range(KD):
                    nc.scalar.dma_start(out=yT_sb[:, kd, :],
                                        in_=yT_r[:, kd, :])

                def attn_tile(t):
                    tsl = slice(t * 128, (t + 1) * 128)
                    s_tok = stok_p.tile([128, DIM], BF16, tag="s_tok")
                    sv = s_tok.rearrange("p (g f) -> p g f", g=2)
                    for grp in range(2):
                        pst = at_ps.tile([128, 4, 128], BF16, tag="atp")
                        for j in range(4):
                            kd = grp * 4 + j
                            nc.tensor.transpose(
                                pst[:, j, :], sT[:, kd, tsl], ident_bf)
                        nc.vector.tensor_copy(out=sv[:, grp, :], in_=pst)
                    gram = asm.tile([128, H * H], F32, tag="gram")
                    for h in range(H):
                        for g in range(h, H):
                            sc_out = scr.tile([128, HD], BF16, tag="sc_out")
                            nc.vector.scalar_tensor_tensor(
                                out=sc_out,
                                in0=s_tok[:, h * HD:(h + 1) * HD],
                                scalar=SCALE,
                                in1=s_tok[:, g * HD:(g + 1) * HD],
                                op0=OP.mult, op1=OP.mult,
                                accum_out=gram[:, h * H + g:h * H + g + 1])
                            if g != h:
                                nc.vector.tensor_copy(
                                    out=gram[:, g * H + h:g * H + h + 1],
                                    in_=gram[:, h * H + g:h * H + g + 1])
                    gv = gram.rearrange("p (h g) -> p h g", h=H)
                    mx = asm.tile([128, H], F32, tag="mx")
                    nc.vector.tensor_reduce(out=mx, in_=gv, axis=AX.X,
                                            op=OP.max)
                    nmx = asm.tile([128, H], F32, tag="nmx")
                    nc.vector.tensor_scalar_mul(nmx, mx, -1.0)
                    pexp = asm.tile([128, H * H], F32, tag="pexp")
                    den = asm.tile([128, H], F32, tag="den")
                    for h in range(H):
                        nc.scalar.activation(
                            out=pexp[:, h * H:(h + 1) * H],
                            in_=gram[:, h * H:(h + 1) * H], func=AF.Exp,
                            bias=nmx[:, h:h + 1], scale=1.0,
                            accum_out=den[:, h:h + 1])
                    rden = asm.tile([128, H], F32, tag="rden")
                    nc.vector.reciprocal(out=rden, in_=den)
                    o_tok = otok_p.tile([128, DIM], BF16, tag="o_tok")
                    for h in range(H):
                        comb = scr.tile([128, HD], F32, tag="comb")
                        nc.vector.tensor_scalar_mul(
                            comb, s_tok[:, 0:HD], pexp[:, h * H:h * H + 1])
                        for g in range(1, H):
                            nc.vector.scalar_tensor_tensor(
                                out=comb, in0=s_tok[:, g * HD:(g + 1) * HD],
                                scalar=pexp[:, h * H + g:h * H + g + 1],
                                in1=comb, op0=OP.mult, op1=OP.add)
                        nc.vector.tensor_scalar_mul(
                            o_tok[:, h * HD:(h + 1) * HD], comb,
                            rden[:, h:h + 1])
                    return o_tok

                def o_transpose(t, o_tok):
                    tsl = slice(t * 128, (t + 1) * 128)
                    for grp in range(2):
                        pst = at_ps.tile([128, 4, 128], BF16, tag="atp")
                        for j in range(4):
                            kd = grp * 4 + j
                            nc.tensor.transpose(
                                pst[:, j, :],
                                o_tok[:, kd * 128:(kd + 1) * 128], ident_bf)
                        nc.vector.tensor_copy(
                            out=oT[:, grp * 4:(grp + 1) * 4, tsl], in_=pst)

                def fc1_cols(mh):
                    if FC1_FP8:
                        w1t = w1str.tile([128, PAIRS, 2, 128], FP8, tag="w1t")
                        for pair in range(PAIRS):
                            nc.sync.dma_start(
                                out=w1t[:, pair],
                                in_=w1_d.ap()[pair, :, :,
                                              mh * 128:(mh + 1) * 128]
                                .rearrange("two p f -> p two f"))
                        for ch in range(NCH):
                            csl = slice(ch * CHUNK, (ch + 1) * CHUNK)
                            psf = f1_ps.tile([128, CHUNK], F32, tag="psf")
                            for pair in range(PAIRS):
                                nc.tensor.matmul(
                                    psf, w1t[:, pair],
                                    nyT8[:, pair, ch],
                                    start=(pair == 0),
                                    stop=(pair == PAIRS - 1), perf_mode=DR)
                            hdst = hT[:, mh // 2, ch, mh % 2, :] \
                                if FC2_FP8 else hT[:, mh, csl]
                            nc.scalar.activation(
                                out=hdst, in_=psf, func=AF.Gelu,
                                bias=bm1_sb[:, mh:mh + 1], scale=fdesc)
                    else:
                        w1t = w1str.tile([128, KD, 128], BF16, tag="w1t")
                        for half in range(2):
                            hs = slice(half * 4, (half + 1) * 4)
                            nc.sync.dma_start(
                                out=w1t[:, hs, :],
                                in_=w1_d.ap().rearrange(
                                    "pr two p f -> (pr two) p f")
                                [hs, :, mh * 128:(mh + 1) * 128]
                                .rearrange("k p f -> p k f"))
                        for ch in range(NCH):
                            csl = slice(ch * CHUNK, (ch + 1) * CHUNK)
                            psf = f1_ps.tile([128, CHUNK], F32, tag="psf")
                            for kd in range(KD):
                                nc.tensor.matmul(
                                    psf, w1t[:, kd, :], nyT[:, kd, csl],
                                    start=(kd == 0), stop=(kd == KD - 1))
                            nc.scalar.activation(
                                out=hT[:, mh, csl], in_=psf, func=AF.Gelu,
                                bias=bm1_sb[:, mh:mh + 1], scale=1.0)

                # 4-tile halves: [attn 0-3][fc1 0-15][attn 4-7][fc1 16-31]
                # -> only 2 Exp<->Gelu ACT table switches on the scalar engine
                for half in range(2):
                    otoks = []
                    for t in range(half * 4, half * 4 + 4):
                        otoks.append((t, attn_tile(t)))
                    for mh in range(half * 16, half * 16 + 16):
                        fc1_cols(mh)
                    for t, o_tok in otoks:
                        o_transpose(t, o_tok)

            # ---------- phase 4: proj + fc2 shared accumulation ----------
            with ExitStack() as p4:
                w2str = p4.enter_context(tc.tile_pool(name="w2str", bufs=6))
                ostg = p4.enter_context(tc.tile_pool(name="ostg", bufs=4))
                f2_ps = p4.enter_context(
                    tc.tile_pool(name="f2_ps", bufs=1, space="PSUM"))

                for g2i in range(4):        # groups of 2 mf tiles
                    ps2 = [[f2_ps.tile([128, CHUNK], F32, tag=f"f2_{m}_{ch}",
                                       name=f"f2_{m}_{ch}_{g2i}")
                            for ch in range(NCH)] for m in range(2)]
                    cols = slice(g2i * 256, (g2i + 1) * 256)
                    for kd in range(KD):
                        wpt = w2str.tile([128, 256], BF16, tag="wpt")
                        nc.sync.dma_start(out=wpt, in_=wp_d.ap()[kd, :, cols])
                        for m in range(2):
                            for ch in range(NCH):
                                csl = slice(ch * CHUNK, (ch + 1) * CHUNK)
                                nc.tensor.matmul(
                                    ps2[m][ch], wpt[:, m * 128:(m + 1) * 128],
                                    oT[:, kd, csl],
                                    start=(kd == 0), stop=False)
                    if FC2_FP8:
                        for khp in range(MH // 2):
                            w2t = w2str.tile([128, 2, 2, 128], FP8, tag="w2t")
                            nc.sync.dma_start(
                                out=w2t,
                                in_=w2_d.ap()[khp, :, 2 * g2i:2 * g2i + 2])
                            for m in range(2):
                                for ch in range(NCH):
                                    csl = slice(ch * CHUNK, (ch + 1) * CHUNK)
                                    nc.tensor.matmul(
                                        ps2[m][ch], w2t[:, m],
                                        hT[:, khp, ch],
                                        start=False, stop=(khp == MH // 2 - 1),
                                        perf_mode=DR)
                    else:
                        for kh in range(MH):
                            w2t = w2str.tile([128, 256], BF16, tag="w2t")
                            nc.sync.dma_start(out=w2t,
                                              in_=w2_d.ap()[kh, :, cols])
                            for m in range(2):
                                for ch in range(NCH):
                                    csl = slice(ch * CHUNK, (ch + 1) * CHUNK)
                                    nc.tensor.matmul(
                                        ps2[m][ch],
                                        w2t[:, m * 128:(m + 1) * 128],
                                        hT[:, kh, csl],
                                        start=False, stop=(kh == MH - 1))
                    # evict: + bias + y^T residual, stream out^T to DRAM
                    for m in range(2):
                        mf = g2i * 2 + m
                        for ch in range(NCH):
                            csl = slice(ch * CHUNK, (ch + 1) * CHUNK)
                            og = ostg.tile([128, CHUNK], F32, tag="og")
                            if FC2_FP8:
                                nc.vector.scalar_tensor_tensor(
                                    out=og, in0=ps2[m][ch],
                                    scalar=1.0 / WSC,
                                    in1=yT_sb[:, mf, csl],
                                    op0=OP.mult, op1=OP.add)
                                nc.vector.tensor_scalar(
                                    out=og, in0=og,
                                    scalar1=bpb2_sb[:, mf:mf + 1],
                                    scalar2=None, op0=OP.add)
                            else:
                                nc.vector.scalar_tensor_tensor(
                                    out=og, in0=ps2[m][ch],
                                    scalar=bpb2_sb[:, mf:mf + 1],
                                    in1=yT_sb[:, mf, csl],
                                    op0=OP.add, op1=OP.add)
                            nc.sync.dma_start(out=outT_r[mf][:, csl], in_=og)

    return nc


MAX_WAITS = 1


def split_big_waits(nc, limit=MAX_WAITS):
    """Walrus rejects instructions carrying too many sem waits; move the
    overflow onto preceding single-wait NoOps on the same engine."""
    n = 0
    for fn in nc.m.functions:
        for blk in fn.blocks:
            new_insts = []
            for inst in blk.instructions:
                si = inst.sync_info
                if si is not None and len(si.on_wait) > limit:
                    waits = list(si.on_wait)
                    while len(waits) > limit:
                        w, waits = waits[0], waits[1:]
                        nop = mybir.InstNoOp(name=f"WSPLIT-{nc.next_id()}")
                        nop.engine = inst.engine
                        nop.sync_info = mybir.SyncInfo(on_wait=[w], on_update=[])
                        new_insts.append(nop)
                        n += 1
                    si.on_wait = waits
                new_insts.append(inst)
            blk.instructions[:] = new_insts
    return n


_NC_CACHE = {}


def get_nc(b_c=B_C):
    """Build + apply the walrus wait-split workaround (HW compile path)."""
    if b_c not in _NC_CACHE:
        nc = build_kernel(b_c)
        split_big_waits(nc)
        _NC_CACHE[b_c] = nc
    return _NC_CACHE[b_c]


def make_in_maps(inputs, b_c=B_C, ncores=NCORES):
    w = prep_weights(
        np.asarray(inputs["Wg"]), np.asarray(inputs["bg"]),
        np.asarray(inputs["Wqkv"]), np.asarray(inputs["Wp"]),
        np.asarray(inputs["bp"]), np.asarray(inputs["g1"]),
        np.asarray(inputs["bn1"]), np.asarray(inputs["g2"]),
        np.asarray(inputs["bn2"]), np.asarray(inputs["W1"]),
        np.asarray(inputs["bm1"]), np.asarray(inputs["W2"]),
        np.asarray(inputs["bm2"]))
    x = f32(np.asarray(inputs["x"]))
    y = f32(np.asarray(inputs["y"]))
    in_maps = []
    for c in range(ncores):
        sl = slice(c * b_c, (c + 1) * b_c)
        in_maps.append({
            "xT": np.ascontiguousarray(x[sl].T),
            "y": y[sl],
            "yT": np.ascontiguousarray(y[sl].T),
            **w,
        })
    return in_maps


def kernel(**inputs):
    nc = get_nc(B_C)
    in_maps = make_in_maps(inputs)
    res = run_bass_kernel_spmd(nc, in_maps, core_ids=list(range(NCORES)))
    return np.concatenate(
        [np.ascontiguousarray(res.results[c]["outT"].T) for c in range(NCORES)],
        axis=0)

